# revision 1
# baseline (speedup 1.0000x reference)
"""Trainium2 Bass kernel for CRF forward-algorithm loss (logsumexp scan).

Exp-domain matmul recurrence (see kernel_v1.py docstring for the math):
    u_t = exp(emit_t - C) * (P @ u_{t-1}),  P = exp(trans), u kept [tags, batch]

v2 optimizations over v1:
  - Emissions DMA'd 8 steps per transfer, exp() applied per 8-step tile.
  - The 16 batch lanes per core are split into two groups of 8 whose matmuls
    are interleaved per weight chunk so both share one LDWEIGHTS stream (the
    PE bottleneck is streaming the 512x512 transition matrix into the array
    every step); duplicate LDWEIGHTS are deleted post-schedule.
  - Per-group PSUM banks + per-group multiplies so DVE work on one group
    hides under the other group's matmuls.
  - Renorm every 16 steps (exact bookkeeping via stored z, log on host).

Sharding: data-parallel over batch, 16 per core on 8 cores, host sums.
"""

import numpy as np
import ml_dtypes

import concourse.bass as bass
import concourse.mybir as mybir
import concourse.tile as tile
from concourse import bacc
from concourse.bass_utils import run_bass_kernel_spmd

T = 512
S = 512
B = 128
NCORES = 8
BL = B // NCORES   # 16 per core
G = 8              # batch per group (2 groups)
TC = 4
START = 510
STOP = 511
C = 7.0
R = 16
NREN = S // R      # 32
DG = 8             # steps per DMA group

F32 = mybir.dt.float32
BF16 = mybir.dt.bfloat16


def _dedup_ldweights(nc):
    removed = 0
    for blk in nc.m.functions[0].blocks:
        insts = blk.instructions
        last_w = None
        to_del = []
        for inst in insts:
            tn = type(inst).__name__
            if tn == "InstLdweights":
                sig = repr(inst.ins[0])
                si = inst.sync_info
                clean = si is None or (
                    len(si.on_wait) == 0 and len(si.on_update) == 0
                )
                if sig == last_w and clean:
                    to_del.append(inst)
                else:
                    last_w = sig
        for inst in to_del:
            insts.remove(inst)
            removed += 1
    return removed


def _build_program():
    nc = bacc.Bacc(
        "TRN2",
        target_bir_lowering=False,
        debug=False,
        enable_asserts=False,
        num_devices=NCORES,
    )

    pt_d = nc.dram_tensor("pt", [128, TC * TC * 128], BF16, kind="ExternalInput")
    pstop_d = nc.dram_tensor("pstop", [128, TC], BF16, kind="ExternalInput")
    u0_d = nc.dram_tensor("u0", [128, TC * G], BF16, kind="ExternalInput")
    em_d = nc.dram_tensor("emt", [S // DG, 128, DG * TC * 2 * G], F32,
                          kind="ExternalInput")
    fin_d = nc.dram_tensor("fin", [1, BL], F32, kind="ExternalOutput")
    zs_d = nc.dram_tensor("zs", [1, NREN * BL], F32, kind="ExternalOutput")

    with tile.TileContext(nc) as tc:
        with (
            tc.tile_pool(name="singles", bufs=1) as singles,
            tc.tile_pool(name="empool", bufs=3) as empool,
            tc.tile_pool(name="ehpool", bufs=3) as ehpool,
            tc.tile_pool(name="upool", bufs=2) as upool,
            tc.tile_pool(name="rnpool", bufs=2) as rnpool,
            tc.tile_pool(name="pspool", bufs=2, space="PSUM") as pspool,
            tc.tile_pool(name="pzpool", bufs=2, space="PSUM") as pzpool,
        ):
            ptsb = singles.tile([128, TC * TC * 128], BF16)
            nc.sync.dma_start(out=ptsb, in_=pt_d[:, :])
            pstop_sb = singles.tile([128, TC], BF16)
            nc.sync.dma_start(out=pstop_sb, in_=pstop_d[:, :])
            uA = upool.tile([128, TC * G], BF16, name="uA", tag="uA")
            nc.sync.dma_start(out=uA, in_=u0_d[:, :])
            uB = upool.tile([128, TC * G], BF16, name="uB", tag="uB")
            nc.sync.dma_start(out=uB, in_=u0_d[:, :])
            ones_sb = singles.tile([128, 1], BF16)
            nc.vector.memset(ones_sb, 1.0)
            negc_sb = singles.tile([128, 1], F32)
            nc.vector.memset(negc_sb, -C)
            zs_sb = singles.tile([1, NREN * BL], F32)

            eh8 = None
            for t in range(S):
                s = t % DG
                if s == 0:
                    gi = t // DG
                    em8 = empool.tile([128, DG * 64], F32, name="em8", tag="em")
                    nc.sync.dma_start(out=em8, in_=em_d[gi])
                    eh8 = ehpool.tile([128, DG * 64], F32, name="eh8", tag="eh")
                    nc.scalar.activation(
                        eh8, em8, mybir.ActivationFunctionType.Exp,
                        bias=negc_sb, scale=1.0,
                    )
                ehv = eh8.rearrange("p (s i g b) -> p s i g b", s=DG, i=TC, g=2)

                psA = pspool.tile([128, TC * G], F32, name="psA", tag="sa")
                psB = pspool.tile([128, TC * G], F32, name="psB", tag="sb")
                for j in range(TC):
                    for i in range(TC):
                        w = ptsb[:, (i * TC + j) * 128 : (i * TC + j + 1) * 128]
                        nc.tensor.matmul(
                            psA[:, j * G : (j + 1) * G], w,
                            uA[:, i * G : (i + 1) * G],
                            start=(i == 0), stop=(i == TC - 1),
                            skip_group_check=True,
                        )
                        nc.tensor.matmul(
                            psB[:, j * G : (j + 1) * G], w,
                            uB[:, i * G : (i + 1) * G],
                            start=(i == 0), stop=(i == TC - 1),
                            skip_group_check=True,
                        )
                uA_new = upool.tile([128, TC * G], BF16, name="uA", tag="uA")
                uB_new = upool.tile([128, TC * G], BF16, name="uB", tag="uB")
                nc.vector.tensor_mul(uA_new, psA, ehv[:, s, :, 0, :])
                nc.vector.tensor_mul(uB_new, psB, ehv[:, s, :, 1, :])

                if t % R == R - 1:
                    r = t // R
                    for g, (u_new, col0) in enumerate(((uA_new, 0), (uB_new, G))):
                        zp = pzpool.tile([1, G], F32, name="zp", tag="z")
                        for i in range(TC):
                            nc.tensor.matmul(
                                zp, ones_sb, u_new[:, i * G : (i + 1) * G],
                                start=(i == 0), stop=(i == TC - 1),
                                skip_group_check=True,
                            )
                        nc.vector.tensor_copy(
                            zs_sb[0:1, r * BL + col0 : r * BL + col0 + G], zp
                        )
                        zr = rnpool.tile([1, G], F32, name="zr", tag="zr")
                        nc.vector.reciprocal(zr, zp)
                        zb = rnpool.tile([128, G], F32, name="zb", tag="zb")
                        nc.gpsimd.partition_broadcast(zb, zr)
                        for i in range(TC):
                            nc.vector.tensor_mul(
                                u_new[:, i * G : (i + 1) * G],
                                u_new[:, i * G : (i + 1) * G], zb,
                            )
                uA, uB = uA_new, uB_new

            fin_sb = singles.tile([1, BL], F32)
            for g, (u, col0) in enumerate(((uA, 0), (uB, G))):
                finp = pzpool.tile([1, G], F32, name="finp", tag="z")
                for i in range(TC):
                    nc.tensor.matmul(
                        finp, pstop_sb[:, i : i + 1], u[:, i * G : (i + 1) * G],
                        start=(i == 0), stop=(i == TC - 1),
                        skip_group_check=True,
                    )
                nc.vector.tensor_copy(fin_sb[0:1, col0 : col0 + G], finp)
            nc.sync.dma_start(out=fin_d[0:1, :], in_=fin_sb)
            nc.sync.dma_start(out=zs_d[0:1, :], in_=zs_sb)

    n = _dedup_ldweights(nc)
    nc._ldw_removed = n
    nc.compile()
    return nc


def _prep_inputs(emissions, transitions):
    bf = ml_dtypes.bfloat16
    P = np.exp(transitions.astype(np.float32))
    PT = np.ascontiguousarray(P.T)                      # [prev, next]
    pt_host = np.ascontiguousarray(
        PT.reshape(TC, 128, TC, 128).transpose(1, 0, 2, 3)
    ).reshape(128, TC * TC * 128).astype(bf)
    pstop = np.exp(transitions[STOP].astype(np.float32))
    pstop_host = np.ascontiguousarray(pstop.reshape(TC, 128).T).astype(bf)
    u0_host = np.zeros((128, TC * G), dtype=bf)
    u0_host[START % 128, (START // 128) * G : (START // 128 + 1) * G] = 1.0

    in_maps = []
    for c in range(NCORES):
        sh = emissions[c * BL : (c + 1) * BL]           # [BL, S, T]
        # emt[gi, k, ((s, i, g, b))] = sh[g*8+b, 8*gi+s, 128*i+k]
        a = sh.transpose(1, 2, 0)                       # [t, n, bb]
        a = a.reshape(S // DG, DG, TC, 128, 2, G)       # [gi, s, i, k, g, b]
        emt = np.ascontiguousarray(a.transpose(0, 3, 1, 2, 4, 5)).reshape(
            S // DG, 128, DG * TC * 2 * G
        ).astype(np.float32)
        in_maps.append({"pt": pt_host, "pstop": pstop_host, "u0": u0_host,
                        "emt": emt})
    return in_maps


def _loss_from_outputs(results):
    total = 0.0
    for res in results:
        fin = np.asarray(res["fin"], np.float64).reshape(BL)
        zs = np.asarray(res["zs"], np.float64).reshape(NREN, BL)
        loss_b = np.log(fin) + np.log(zs).sum(axis=0) + S * C
        total += loss_b.sum()
    return np.float32(total)


def _run(inputs, **kwargs):
    emissions = np.asarray(inputs["inputs"], dtype=np.float32)
    transitions = np.asarray(inputs["transitions"], dtype=np.float32)
    assert emissions.shape == (B, S, T), emissions.shape
    nc = _build_program()
    in_maps = _prep_inputs(emissions, transitions)
    res = run_bass_kernel_spmd(nc, in_maps, core_ids=list(range(NCORES)), **kwargs)
    return _loss_from_outputs(res.results), res


def kernel(**inputs) -> np.ndarray:
    out, _ = _run(inputs)
    return out

